# revision 18
# baseline (speedup 1.0000x reference)
"""Trainium2 Bass kernel for nn_BasicEncoder (embedding-lookup encoder).

reference math:
    counts[b, v] = histogram of the 512 token ids in row b          [B, V]
    h      = relu(counts @ enc1_w.T + enc1_b)                       [B, 16]
    mean   = h @ mean_w.T + mean_b                                  [B, 16]
    logvar = h @ logvar_w.T + logvar_b                              [B, 16]

Key identity: counts @ enc1_w.T  ==  sum_s enc1_w[:, x[b, s]], i.e. a
gather-and-sum of embedding-table columns.  The [B, V] histogram is never
materialized.

Device strategy (data-parallel over 8 NeuronCores, batch-sharded, 256
rows x 512 tokens per core):

  - The [16, V] table is replicated 8x down the SBUF partitions
    ([128, V] f32, 125.5KB/partition).  The GpSimd ap_gather instruction
    gathers per 16-partition group with that group's own index stream,
    so 8 batch rows are gathered concurrently (one per Q7 core):
        gath[16g + h, j*512 + s] = enc1_w[h, x[row(g, j), s]]
  - A vector-engine reduce over the free dim sums each row's 512
    embeddings: hall[16g + h, n] = h[row(g, n), h].
  - enc1_b rides the scalar-engine Relu as a per-partition bias
    (partition p has h-dim p%16).
  - mean/logvar = one matmul each against a block-diagonal (8 x [16,16])
    weight matrix, keeping the (group, h) partition layout; output biases
    added as per-partition scalars.  Host unscrambles [128, 32] -> [256, 32].
"""

import numpy as np

B, S, V, H, O = 2048, 512, 32128, 16, 16
NCORES = 8
RPC = B // NCORES  # rows per core (256)
P = 128
G = 8  # partition groups (= Q7 cores)
SLOTS = RPC // G  # rows per group (32)
RPG = 8  # rows gathered per group per ap_gather instruction
NINST = SLOTS // RPG  # ap_gather instructions per core (8)
NI = RPG * S  # indices per group per instruction (2048)
NIW = NI // 16  # wrapped idx columns per instruction (128)

_CACHE = {}


def _build_nc(repeat=1):
    import contextlib

    import concourse.bacc as bacc
    import concourse.bass as bass
    import concourse.mybir as mybir
    import concourse.tile as tile

    f32 = mybir.dt.float32
    i16 = mybir.dt.int16
    nc = bacc.Bacc(None, target_bir_lowering=False)

    tbl_d = nc.dram_tensor("tblr", [H, V], f32, kind="ExternalInput")
    xi_d = nc.dram_tensor("xi16", [P, NINST * NIW], i16, kind="ExternalInput")
    b1_d = nc.dram_tensor("b1rep", [P, 1], f32, kind="ExternalInput")
    wm_d = nc.dram_tensor("wmbd", [P, P], f32, kind="ExternalInput")
    wl_d = nc.dram_tensor("wlbd", [P, P], f32, kind="ExternalInput")
    bm_d = nc.dram_tensor("bmrep", [P, 1], f32, kind="ExternalInput")
    bl_d = nc.dram_tensor("blrep", [P, 1], f32, kind="ExternalInput")
    out_d = nc.dram_tensor("out", [P, 2 * SLOTS], f32, kind="ExternalOutput")

    with tile.TileContext(nc) as tc:
        with (
            tc.tile_pool(name="sb", bufs=1) as pool,
            tc.tile_pool(name="gth", bufs=3) as gpool,
            tc.tile_pool(name="ps", bufs=1, space=bass.MemorySpace.PSUM) as pspool,
            tc.For_i(0, repeat, 1) if repeat > 1 else contextlib.nullcontext(),
        ):
            tbl_sb = pool.tile([P, V], f32)
            for g in range(G):
                nc.sync.dma_start(tbl_sb[g * H : (g + 1) * H, :], tbl_d[:])
            xi_sb = pool.tile([P, NINST * NIW], i16)
            nc.sync.dma_start(xi_sb[:], xi_d[:])
            b1_sb = pool.tile([P, 1], f32)
            nc.sync.dma_start(b1_sb[:], b1_d[:])
            wm_sb = pool.tile([P, P], f32)
            nc.sync.dma_start(wm_sb[:], wm_d[:])
            wl_sb = pool.tile([P, P], f32)
            nc.sync.dma_start(wl_sb[:], wl_d[:])
            bm_sb = pool.tile([P, 1], f32)
            nc.sync.dma_start(bm_sb[:], bm_d[:])
            bl_sb = pool.tile([P, 1], f32)
            nc.sync.dma_start(bl_sb[:], bl_d[:])

            hall = pool.tile([P, SLOTS], f32)
            for k in range(NINST):
                gath = gpool.tile([P, NI], f32)
                nc.gpsimd.ap_gather(
                    out_ap=gath[:],
                    in_ap=tbl_sb[:],
                    idxs_ap=xi_sb[:, k * NIW : (k + 1) * NIW],
                    channels=P,
                    num_elems=V,
                    d=1,
                    num_idxs=NI,
                )
                nc.vector.tensor_reduce(
                    out=hall[:, k * RPG : (k + 1) * RPG],
                    in_=gath[:].rearrange("p (r s) -> p r s", s=S),
                    axis=mybir.AxisListType.X,
                    op=mybir.AluOpType.add,
                )

            hr = pool.tile([P, SLOTS], f32)
            nc.scalar.activation(
                out=hr[:],
                in_=hall[:],
                func=mybir.ActivationFunctionType.Relu,
                bias=b1_sb[:],
            )

            om_ps = pspool.tile([P, SLOTS], f32)
            nc.tensor.matmul(om_ps[:], wm_sb[:], hr[:])
            ol_ps = pspool.tile([P, SLOTS], f32)
            nc.tensor.matmul(ol_ps[:], wl_sb[:], hr[:])

            o_sb = pool.tile([P, 2 * SLOTS], f32)
            nc.vector.tensor_scalar(
                out=o_sb[:, :SLOTS],
                in0=om_ps[:],
                scalar1=bm_sb[:],
                scalar2=None,
                op0=mybir.AluOpType.add,
            )
            nc.vector.tensor_scalar(
                out=o_sb[:, SLOTS:],
                in0=ol_ps[:],
                scalar1=bl_sb[:],
                scalar2=None,
                op0=mybir.AluOpType.add,
            )
            nc.sync.dma_start(out_d[:], o_sb[:])

    nc.compile()
    return nc


def _get_nc(repeat=1):
    key = ("nc", repeat)
    if key not in _CACHE:
        _CACHE[key] = _build_nc(repeat)
    return _CACHE[key]


def _prep_inputs(x, enc1_w, enc1_b, mean_w, mean_b, logvar_w, logvar_b):
    x = np.asarray(x)
    assert x.shape == (B, S)
    # row r of core c = global row c*RPC + r; within a core, row r is
    # handled by group g = r % G at slot n = r // G; instruction k covers
    # slots k*RPG .. k*RPG+RPG-1.
    xs = x.astype(np.int16).reshape(NCORES, SLOTS, G, S)  # [c, n, g, s]
    # per (c, k, g): index stream = concat over j (slot n=k*RPG+j) of tokens
    stream = xs.transpose(0, 2, 1, 3).reshape(NCORES, G, NINST, NI)  # [c,g,k,i]
    # wrapped: idx i -> [16g + i%16, i//16]
    wrapped = stream.reshape(NCORES, G, NINST, NIW, 16).transpose(0, 1, 4, 2, 3)
    xi16 = np.ascontiguousarray(
        wrapped.reshape(NCORES, G * 16, NINST * NIW)
    )  # [c, 128, NINST*NIW]

    tblr = np.ascontiguousarray(np.asarray(enc1_w, dtype=np.float32))  # [H, V]
    pidx = np.arange(P) % H
    b1rep = np.asarray(enc1_b, dtype=np.float32)[pidx][:, None].copy()
    bmrep = np.asarray(mean_b, dtype=np.float32)[pidx][:, None].copy()
    blrep = np.asarray(logvar_b, dtype=np.float32)[pidx][:, None].copy()

    # block-diagonal stationary: w_bd[16g+h, 16g+o] = w[o, h]
    def blockdiag(w):
        w = np.asarray(w, dtype=np.float32)  # [O, H]
        bd = np.zeros((P, P), dtype=np.float32)
        for g in range(G):
            bd[g * H : (g + 1) * H, g * O : (g + 1) * O] = w.T
        return bd

    wmbd = blockdiag(mean_w)
    wlbd = blockdiag(logvar_w)
    return [
        {
            "tblr": tblr,
            "xi16": xi16[c],
            "b1rep": b1rep,
            "wmbd": wmbd,
            "wlbd": wlbd,
            "bmrep": bmrep,
            "blrep": blrep,
        }
        for c in range(NCORES)
    ]


def _unscramble(out_core):
    # out_core [128, 2*SLOTS]: partition 16g+o, col n -> row n*G+g
    o = out_core.reshape(G, O, 2, SLOTS)  # [g, o, {m,l}, n]
    o = o.transpose(2, 3, 0, 1)  # [{m,l}, n, g, o]
    return o.reshape(2, SLOTS * G, O)  # rows r = n*G + g


def _run(in_maps, trace=False, repeat=1):
    from concourse.bass_utils import run_bass_kernel_spmd

    nc = _get_nc(repeat)
    core_ids = list(range(NCORES))
    res = run_bass_kernel_spmd(nc, in_maps, core_ids, trace=trace)
    mean = np.empty((B, O), dtype=np.float32)
    logvar = np.empty((B, O), dtype=np.float32)
    for c in core_ids:
        ml = _unscramble(res.results[c]["out"])
        mean[c * RPC : (c + 1) * RPC] = ml[0]
        logvar[c * RPC : (c + 1) * RPC] = ml[1]
    return mean, logvar, res


def kernel(x, enc1_w, enc1_b, mean_w, mean_b, logvar_w, logvar_b):
    in_maps = _prep_inputs(x, enc1_w, enc1_b, mean_w, mean_b, logvar_w, logvar_b)
    mean, logvar, _ = _run(in_maps, trace=False)
    return mean, logvar

